# revision 1
# baseline (speedup 1.0000x reference)
"""Tucker-style 3-mode contraction kernel for Trainium2 (8 NeuronCores).

Problem: x [1024*32*32*32] fp32, w0/w1/w2 [32,32] fp32.
  out[B,A,Bb,C] = sum_{a,b,c} x[B,a,b,c] w0[a,A] w1[b,Bb] w2[c,C]

Data-parallel over batch: 128 batch elems/core; sub-tile = 4 batch elems
("groups" g) x full 32x32x32 tensor -> [128 p = (g, mode), 1024 f].
Stationary weights are kron(I4, w) [128,128]; one matmul (2x N=512)
contracts the partition-inner mode of all 4 groups at once.

v6 (contract order c, b, a; fp16 intermediates for 1 cyc/row matmuls;
every engine access pattern dense or large-run strided; work spread so
engine maxima sit near the DMA roofline):

  X    [(g,a),(b,c)] f32     <- dense DMA in (super-tile 2 MiB, SP queue)
  C0   cast f16 (dense)      -> xbd [p,(b,c)]       (Pool cols + ACT cols)
  T0   DVE ST                -> xt  [(g,c),(b,a)]
  MM1  kron(w2) f16          -> z1  [(g,C),(b,a)]   psum f32
  D1   ACT reorder+cast      -> z1b [p,(a,b)] f16
  T1   DVE ST                -> z1t [(g,b),(a,C)]
  MM2  kron(w1) f16          -> z2  [(g,B),(a,C)]   psum f32
  D2   ACT pack C-pairs      -> z2b [p, C2, a, Ci] f16
  T2   DVE ST on uint32 pairs [128,512] (2x) -> z2t [(g,a),(C2,B,Ci)]
  MM3  kron(w0) f16          -> z3  [(g,A),(C2,B,Ci)] psum f32
  OUT  ACT unpack -> Y [(g,A),(B,C)] f32 -> dense DMA out (SP queue)
"""

import os

import numpy as np

N_CORES = 8
BATCH = 1024
F = 32  # factor dim
ELEM = F * F * F  # 32768 elems per batch element
B_PER_CORE = BATCH // N_CORES  # 128
G = 4  # batch groups per sub-tile (4*32 = 128 partitions)
S = 4  # sub-tiles per super-tile
T = B_PER_CORE // (G * S)  # 8 super-tiles per core
FF = F * F  # 1024

# intermediate/matmul dtype: "float16" (default) or "bfloat16"
Z_DTYPE = os.environ.get("KERNEL_Z_DTYPE", "float16")
X_DTYPE = Z_DTYPE  # kept for test.py printout compat
# engine split knobs
C0_POOL = int(os.environ.get("KERNEL_C0_POOL", "720"))  # of 1024 cols
OUT_ACT = int(os.environ.get("KERNEL_OUT_ACT", "18"))   # of 32 B-rows

_CACHE = {}


def build_program(z_dtype=Z_DTYPE, repeat=1):
    key = (z_dtype, repeat)
    if key in _CACHE:
        return _CACHE[key]

    import concourse.bacc as bacc
    import concourse.mybir as mybir
    import concourse.tile as tile

    f32 = mybir.dt.float32
    u32 = mybir.dt.uint32
    zdt = getattr(mybir.dt, z_dtype)

    nc = bacc.Bacc("TRN2", target_bir_lowering=False, debug=False,
                   num_devices=N_CORES)

    xs = nc.dram_tensor("xs", [T, S, G, F, FF], f32, kind="ExternalInput")
    wk2 = nc.dram_tensor("wk2", [128, 128], zdt, kind="ExternalInput")
    wk1 = nc.dram_tensor("wk1", [128, 128], zdt, kind="ExternalInput")
    wk0 = nc.dram_tensor("wk0", [128, 128], zdt, kind="ExternalInput")
    ys = nc.dram_tensor("ys", [T, S, G, F, FF], f32, kind="ExternalOutput")

    def mm(out_ap, lhsT_ap, rhs_ap):
        nc.tensor.matmul(out_ap, lhsT_ap, rhs_ap, start=True, stop=True)

    with tile.TileContext(nc) as tc:
        with (
            tc.tile_pool(name="consts", bufs=1) as cpool,
            tc.tile_pool(name="xp", bufs=3) as xp,
            tc.tile_pool(name="xbp", bufs=3) as xbp,
            tc.tile_pool(name="xtp", bufs=3) as xtp,
            tc.tile_pool(name="z1bp", bufs=3) as z1bp,
            tc.tile_pool(name="z1tp", bufs=3) as z1tp,
            tc.tile_pool(name="z2bp", bufs=3) as z2bp,
            tc.tile_pool(name="z2tp", bufs=3) as z2tp,
            tc.tile_pool(name="yp", bufs=2) as yp,
            tc.tile_pool(name="ps1", bufs=1, space="PSUM") as ps1,
            tc.tile_pool(name="ps2", bufs=1, space="PSUM") as ps2,
            tc.tile_pool(name="ps3", bufs=2, space="PSUM") as ps3,
        ):
            wk2t = cpool.tile([128, 128], zdt)
            wk1t = cpool.tile([128, 128], zdt)
            wk0t = cpool.tile([128, 128], zdt)
            nc.sync.dma_start(out=wk2t[:], in_=wk2[:])
            nc.sync.dma_start(out=wk1t[:], in_=wk1[:])
            nc.sync.dma_start(out=wk0t[:], in_=wk0[:])

            for t in range(T * repeat):
                t = t % T
                X = xp.tile([128, S, FF], f32)  # [(g,a), s, (b,c)]
                for si in range(S):
                    nc.sync.dma_start(
                        out=X[:, si],
                        in_=xs[t, si].rearrange("g a m -> (g a) m"))
                Y = yp.tile([128, S, F, F], f32)  # [(g,A), s, B, C]
                for s in range(S):
                    # C0: dense cast f32 -> f16 (Pool cols + ACT cols)
                    xbd = xbp.tile([128, FF], zdt, tag="xbd")
                    nc.gpsimd.tensor_copy(
                        out=xbd[:, 0:832], in_=X[:, s, 0:832])
                    nc.scalar.copy(
                        out=xbd[:, 832:FF], in_=X[:, s, 832:FF])
                    # T0: [(g,a),(b,c)] -> [(g,c),(b,a)]
                    xt = xtp.tile([128, FF], zdt, tag="xt")
                    nc.vector.transpose(out=xt[:], in_=xbd[:])
                    # MM1: contract c -> z1 [(g,C),(b,a)]
                    z1 = ps1.tile([128, FF], f32, tag="z1")
                    mm(z1[:, 0:512], wk2t[:], xt[:, 0:512])
                    mm(z1[:, 512:1024], wk2t[:], xt[:, 512:1024])
                    # D1: reorder (b,a)->(a,b) + cast -> z1b [p,(a,b)]
                    z1b = z1bp.tile([128, F, F], zdt, tag="z1b")
                    nc.scalar.copy(
                        out=z1b[:],
                        in_=z1[:].rearrange("p (b a) -> p a b", b=F, a=F))
                    # T1: -> z1t [(g,b),(a,C)]
                    z1t = z1tp.tile([128, FF], zdt, tag="z1t")
                    nc.vector.transpose(
                        out=z1t[:], in_=z1b[:].rearrange("p a b -> p (a b)"))
                    # MM2: contract b -> z2 [(g,B),(a,C)]
                    z2 = ps2.tile([128, FF], f32, tag="z2")
                    mm(z2[:, 0:512], wk1t[:], z1t[:, 0:512])
                    mm(z2[:, 512:1024], wk1t[:], z1t[:, 512:1024])
                    # D2: pack C-pairs: -> z2b [p, C2, a, Ci]
                    z2b = z2bp.tile([128, F // 2, F, 2], zdt, tag="z2b")
                    nc.scalar.copy(
                        out=z2b[:],
                        in_=z2[:].rearrange("p (a c2 ci) -> p c2 a ci",
                                            a=F, c2=F // 2, ci=2))
                    # T2: packed u32 ST -> z2t [(g,a), (C2, B)] (pairs Ci)
                    z2t = z2tp.tile([128, 512], u32, tag="z2t")
                    nc.vector.transpose(
                        out=z2t[:],
                        in_=z2b[:].rearrange("p c2 a ci -> p (c2 a ci)")
                        .bitcast(u32))
                    # MM3: contract a -> z3 [(g,A), (C2, B, Ci)]
                    z2tv = z2t[:].bitcast(zdt)
                    z3 = ps3.tile([128, FF], f32, tag="z3")
                    mm(z3[:, 0:512], wk0t[:], z2tv[:, 0:512])
                    mm(z3[:, 512:1024], wk0t[:], z2tv[:, 512:1024])
                    # OUT: (C2,B,Ci) -> (B,C) -> Y f32 (ACT)
                    nc.scalar.copy(
                        out=Y[:, s].rearrange("p b (c2 ci) -> p b c2 ci",
                                              c2=F // 2, ci=2),
                        in_=z3[:].rearrange("p (c2 b ci) -> p b c2 ci",
                                            c2=F // 2, b=F, ci=2))
                    nc.sync.dma_start(
                        out=ys[t, s].rearrange("g a (b c) -> (g a) b c",
                                               b=F, c=F),
                        in_=Y[:, s])

    nc.compile()
    _CACHE[key] = nc
    return nc


def _kron4(w, np_dtype):
    return np.kron(np.eye(G, dtype=np.float32),
                   np.asarray(w, np.float32)).astype(np_dtype)


def make_in_maps(x, w0, w1, w2, z_dtype=Z_DTYPE):
    import ml_dtypes
    zdt_np = np.dtype(ml_dtypes.bfloat16) if z_dtype == "bfloat16" \
        else np.dtype(np.float16)
    x = np.ascontiguousarray(np.asarray(x, np.float32).reshape(-1))
    assert x.size == BATCH * ELEM
    shards = x.reshape(N_CORES, T, S, G, F, FF)
    wk2 = _kron4(w2, zdt_np)
    wk1 = _kron4(w1, zdt_np)
    wk0 = _kron4(w0, zdt_np)
    return [
        {"xs": shards[i], "wk2": wk2, "wk1": wk1, "wk0": wk0}
        for i in range(N_CORES)
    ]


def kernel(x, w0, w1, w2, trace=False):
    from concourse.bass_utils import run_bass_kernel_spmd

    nc = build_program()
    in_maps = make_in_maps(x, w0, w1, w2)
    res = run_bass_kernel_spmd(nc, in_maps, core_ids=list(range(N_CORES)),
                               trace=trace)
    out = np.concatenate([res.results[i]["ys"].reshape(-1)
                          for i in range(N_CORES)])
    if trace:
        return out, res
    return out



# revision 14
# speedup vs baseline: 1.8362x; 1.8362x over previous
"""Tucker-style 3-mode contraction kernel for Trainium2 (8 NeuronCores).

Problem: x [1024*32*32*32] fp32, w0/w1/w2 [32,32] fp32.
  out[B,A,Bb,C] = sum_{a,b,c} x[B,a,b,c] w0[a,A] w1[b,Bb] w2[c,C]

v8: bf16 I/O (host casts), contract order a -> c -> b.
Per core: 128 batch elems as 32 sub-tiles of [128 p = (g4, mode32), 1024 f].
Stationary weights kron(I4, w) [128,128] bf16.

Per sub-tile (T1_MODE="hi", default):
  DMA in  X [(g,a),(b,c)] bf16                 (natural layout)
  MM1 wk0 -> ps1 [(g,A),(b,c)] f32             (2x N=512)
  T1  DVE f32 stream-transpose PSUM->SBUF -> t1 [(g,c),(b,A)] f32
  MM2 wk2, rhs = high-16-bit halves of t1 viewed as bf16 (stride-2 AP;
      truncation cast for free) -> ps2 [(g,C),(b,A)] f32
  E2  ACT reorder+cast -> t2 [(g,C),(Ah,b,Ap)] bf16    (A = 2*Ah+Ap)
  T2  DVE u32-pair transpose -> t2t [(g,b),(Ah,C,Ap)] bf16
  MM3 wk1 -> ps3 [(g,B),(Ah,C,Ap)] f32
  E3  cast evac -> Y bf16 (ACT/DVE col split via KERNEL_E3_DVE)
  DMA out                                       (host unscrambles Ah/Ap)

T1_MODE="u32" fallback: E1 ACT strided cast evac to (bh,c,bp) pair
layout + u32-pair transpose instead of the fused f32 transpose.
"""

import os

import numpy as np

N_CORES = 8
BATCH = 1024
F = 32
FF = F * F  # 1024
ELEM = F * FF  # 32768
B_PER_CORE = BATCH // N_CORES  # 128
G = 4  # batch groups per sub-tile
NT = B_PER_CORE // G  # 32 sub-tiles per core
SS = 4  # sub-tiles per super-tile (DMA batch)
NST = NT // SS  # 8 super-tiles per core

T1_MODE = os.environ.get("KERNEL_T1_MODE", "hi")  # "hi" | "u32"
E3_DVE = int(os.environ.get("KERNEL_E3_DVE", "0"))  # cols on DVE (of 1024)

X_DTYPE = Z_DTYPE = "bfloat16"  # for test.py printout compat

_CACHE = {}


def build_program():
    key = (T1_MODE, E3_DVE)
    if key in _CACHE:
        return _CACHE[key]

    import concourse.bacc as bacc
    import concourse.mybir as mybir
    import concourse.tile as tile

    f32 = mybir.dt.float32
    u32 = mybir.dt.uint32
    bf16 = mybir.dt.bfloat16

    nc = bacc.Bacc("TRN2", target_bir_lowering=False, debug=False,
                   num_devices=N_CORES)

    xs = nc.dram_tensor("xs", [NT, 128, FF], bf16, kind="ExternalInput")
    wk0 = nc.dram_tensor("wk0", [128, 128], bf16, kind="ExternalInput")
    wk1 = nc.dram_tensor("wk1", [128, 128], bf16, kind="ExternalInput")
    wk2 = nc.dram_tensor("wk2", [128, 128], bf16, kind="ExternalInput")
    ys = nc.dram_tensor("ys", [NT, 128, FF], bf16, kind="ExternalOutput")

    def mm(out_ap, lhsT_ap, rhs_ap):
        nc.tensor.matmul(out_ap, lhsT_ap, rhs_ap, start=True, stop=True)

    with tile.TileContext(nc) as tc:
        with (
            tc.tile_pool(name="consts", bufs=1) as cpool,
            tc.tile_pool(name="xp", bufs=3) as xp,
            tc.tile_pool(name="t0p", bufs=2) as t0p,
            tc.tile_pool(name="t1p", bufs=2) as t1p,
            tc.tile_pool(name="t2p", bufs=2) as t2p,
            tc.tile_pool(name="t2tp", bufs=2) as t2tp,
            tc.tile_pool(name="yp", bufs=2) as yp,
            tc.tile_pool(name="ps1", bufs=1, space="PSUM") as ps1,
            tc.tile_pool(name="ps2", bufs=1, space="PSUM") as ps2,
            tc.tile_pool(name="ps3", bufs=2, space="PSUM") as ps3,
        ):
            wk0t = cpool.tile([128, 128], bf16)
            wk1t = cpool.tile([128, 128], bf16)
            wk2t = cpool.tile([128, 128], bf16)
            nc.sync.dma_start(out=wk0t[:], in_=wk0[:])
            nc.sync.dma_start(out=wk1t[:], in_=wk1[:])
            nc.sync.dma_start(out=wk2t[:], in_=wk2[:])

            for st in range(NST):
                X = xp.tile([128, SS, FF], bf16, tag="X")
                nc.sync.dma_start(
                    out=X[:],
                    in_=xs[st * SS:(st + 1) * SS].rearrange("t p f -> p t f"))
                Y = yp.tile([128, SS, FF], bf16, tag="Y")
                for s in range(SS):
                    # MM1: contract a -> ps1 [(g,A),(b,c)]
                    z1 = ps1.tile([128, FF], f32, tag="z1")
                    mm(z1[:, 0:512], wk0t[:], X[:, s, 0:512])
                    mm(z1[:, 512:1024], wk0t[:], X[:, s, 512:1024])
                    if T1_MODE == "hi":
                        # T1: f32 psum->sbuf transpose -> [(g,c),(b,A)] f32
                        t1 = t1p.tile([128, FF], f32, tag="t1")
                        nc.vector.transpose(out=t1[:], in_=z1[:])
                        # MM2 rhs: high bf16 halves of f32 (truncation cast)
                        t1v = t1[:].bitcast(bf16).rearrange(
                            "p (b a two) -> p b a two", b=F, a=F, two=2)
                        z2 = ps2.tile([128, FF], f32, tag="z2")
                        mm(z2[:, 0:512], wk2t[:], t1v[:, 0:16, :, 1])
                        mm(z2[:, 512:1024], wk2t[:], t1v[:, 16:32, :, 1])
                        # ps2 free layout (b, A): A = 2*Ah + Ap
                        e2_in = z2[:].rearrange(
                            "p (b ah ap) -> p ah b ap", b=F, ah=16, ap=2)
                    else:
                        # E1: ACT cast evac to pair layout (bh, c, bp)
                        t0 = t0p.tile([128, 16, F, 2], bf16, tag="t0")
                        nc.scalar.copy(
                            out=t0[:],
                            in_=z1[:].rearrange(
                                "p (bh bp c) -> p bh c bp",
                                bh=16, bp=2, c=F))
                        # T1u: u32 pair transpose -> [(g,c),(bh,A,bp)]
                        t1 = t1p.tile([128, FF], bf16, tag="t1")
                        nc.vector.transpose(
                            out=t1[:].bitcast(u32),
                            in_=t0[:].rearrange(
                                "p bh c bp -> p (bh c bp)").bitcast(u32))
                        z2 = ps2.tile([128, FF], f32, tag="z2")
                        mm(z2[:, 0:512], wk2t[:], t1[:, 0:512])
                        mm(z2[:, 512:1024], wk2t[:], t1[:, 512:1024])
                        # ps2 free layout (bh, A, bp): b = 2*bh + bp
                        e2_in = z2[:].rearrange(
                            "p (bh ah ap bp) -> p ah (bh bp) ap",
                            bh=16, ah=16, ap=2, bp=2)
                    # E2: reorder+cast -> t2 [(g,C), (Ah, b, Ap)]
                    t2 = t2p.tile([128, 16, F, 2], bf16, tag="t2")
                    nc.scalar.copy(out=t2[:], in_=e2_in)
                    # T2: u32 pair transpose -> [(g,b), (Ah, C, Ap)]
                    t2t = t2tp.tile([128, 512], u32, tag="t2t")
                    nc.vector.transpose(
                        out=t2t[:],
                        in_=t2[:].rearrange("p ah b ap -> p (ah b ap)")
                        .bitcast(u32))
                    # MM3: contract b -> ps3 [(g,B), (Ah, C, Ap)]
                    t2tv = t2t[:].bitcast(bf16)
                    z3 = ps3.tile([128, FF], f32, tag="z3")
                    mm(z3[:, 0:512], wk1t[:], t2tv[:, 0:512])
                    mm(z3[:, 512:1024], wk1t[:], t2tv[:, 512:1024])
                    # E3: cast evac (ACT, optional DVE col share)
                    ca = FF - E3_DVE
                    if ca > 0:
                        nc.scalar.copy(out=Y[:, s, 0:ca], in_=z3[:, 0:ca])
                    if E3_DVE > 0:
                        nc.vector.tensor_copy(
                            out=Y[:, s, ca:FF], in_=z3[:, ca:FF])
                nc.sync.dma_start(
                    out=ys[st * SS:(st + 1) * SS].rearrange("t p f -> p t f"),
                    in_=Y[:])

    nc.compile()
    _CACHE[key] = nc
    return nc


def _kron4(w, np_dtype):
    return np.kron(np.eye(G, dtype=np.float32),
                   np.asarray(w, np.float32)).astype(np_dtype)


def make_in_maps(x, w0, w1, w2):
    import ml_dtypes
    bf = np.dtype(ml_dtypes.bfloat16)
    x = np.ascontiguousarray(np.asarray(x, np.float32).reshape(-1))
    assert x.size == BATCH * ELEM
    shards = x.reshape(N_CORES, NT, 128, FF).astype(bf)
    wk0 = _kron4(w0, bf)
    wk1 = _kron4(w1, bf)
    wk2 = _kron4(w2, bf)
    return [
        {"xs": shards[i], "wk0": wk0, "wk1": wk1, "wk2": wk2}
        for i in range(N_CORES)
    ]


def kernel(x, w0, w1, w2, trace=False):
    from concourse.bass_utils import run_bass_kernel_spmd

    nc = build_program()
    in_maps = make_in_maps(x, w0, w1, w2)
    res = run_bass_kernel_spmd(nc, in_maps, core_ids=list(range(N_CORES)),
                               trace=trace)
    # ys: [NT, (g, B), (Ah, C, Ap)] per core -> out[batch, A, B, C]
    ys = np.stack([res.results[i]["ys"] for i in range(N_CORES)])
    ys = ys.reshape(N_CORES, NT, G, F, 16, F, 2)  # [core,t,g,B,Ah,C,Ap]
    out = ys.transpose(0, 1, 2, 4, 6, 3, 5)       # [core,t,g,Ah,Ap,B,C]
    out = np.ascontiguousarray(out).astype(np.float32).reshape(-1)
    if trace:
        return out, res
    return out
